# revision 1
# baseline (speedup 1.0000x reference)
"""Trainium2 Bass kernel for nn_Mlp_moe (ViT MLP block with MoE-routed cls
tokens), SPMD across 8 NeuronCores.

Sharding:
  - Patch-token MLP (fc1 -> GELU -> fc2): data-parallel over batch
    (8 batches per core). Weights replicated, bf16 compute, fp32 accum.
  - Cls/atom MoE path: hidden-dim sharded (each core owns a 384-wide slice
    of every atom's hidden dim, for all 64 batches); the per-core partial
    outputs (linear in the hidden contributions) are summed with a
    ReduceScatter whose output shards line up with each core's batch slice.
  - Gate (route logits/softmax/argmax): replicated on every core in fp32;
    folded into the atom path as per-route column scales so the hard
    dispatch is just a sum.
"""

import numpy as np
import ml_dtypes

import bass_rust
import concourse.bass as bass
import concourse.mybir as mybir
import concourse.tile as tile
from concourse.bass_utils import run_bass_kernel_spmd
from concourse.masks import make_identity
from concourse.vector_clock import ScopedClock

F32 = mybir.dt.float32
BF16 = mybir.dt.bfloat16
AF = mybir.ActivationFunctionType
ALU = mybir.AluOpType

N_CORES = 8
B, T, D, H = 64, 203, 768, 3072
NCLS, NP, NA = 6, 197, 5
BC = B // N_CORES          # batches per core
TOK = BC * T               # 1624 tokens per core (cls + patch)
HC = H // N_CORES          # 384 hidden slice per core (cls path)
NTOK_CLS = B * NCLS        # 384 cls tokens globally
TT = 4                     # token tiles for the MLP
TTOK = TOK // TT           # 406

ATOM = {'vm': 0, 'im': 1, 'cm': 2, 'sc': 3, 'cc': 4}
TASK_PAIRS = [('vm', 'sc'), ('vm', 'cc'), ('im', 'sc'), ('im', 'cc'),
              ('cm', 'sc'), ('cm', 'cc')]
SRC = [[ATOM[l], ATOM[r]] for l, r in TASK_PAIRS]
DST = [[ATOM[r], ATOM[l]] for l, r in TASK_PAIRS]


# ---------------------------------------------------------------------------
# Walrus in this container accepts at most ONE sync-wait per instruction.
# Tile emits multi-wait instructions; split the extras onto preceding
# same-engine wait-nops (engines execute in order, semantics preserved).
# ---------------------------------------------------------------------------

def _patched_drain_and_barrier(self, tick_clock, wait_clock):
    nc = self.nc
    drain_inst = nc.sync.drain()
    wait_clock.add_sem_waits(
        drain_inst.ins, ScopedClock({None: tick_clock.global_clock}))
    si = drain_inst.ins.sync_info
    waits = list(si.on_wait) if si is not None and si.on_wait else []
    if len(waits) > 1:
        drain_inst.ins.sync_info = bass_rust.SyncInfo(
            on_wait=waits[:1], on_update=list(si.on_update or []))
        for w in waits[1:]:
            nop = nc.sync.nop(nofuse=True, hint="drain_wait_split")
            nop.ins.sync_info = bass_rust.SyncInfo(on_wait=[w], on_update=[])
    nc.all_engine_barrier()
    assert self.sems is not None
    popped = nc._tile_sem_poison_stack.pop()
    assert popped is self._sem_poison
    nc.clear_and_free_semaphores(list(self.sems.allocated().values()))
    nc.all_engine_barrier()


tile.TileContext._drain_and_barrier = _patched_drain_and_barrier


def legalize_sync_waits(nc):
    n_split = 0
    for f in nc.m.functions:
        for bb in f.blocks:
            insts = bb.instructions
            new_list = []
            for inst in insts:
                si = inst.sync_info
                waits = list(si.on_wait) if si is not None and si.on_wait else []
                if len(waits) > 1:
                    for w in waits[1:]:
                        eng = nc.engines[inst.engine]
                        nop = eng.nop(nofuse=True, hint="wait_split")
                        cur = nc.cur_bb.bb.instructions
                        assert cur and cur[-1] is nop.ins
                        cur.pop()
                        nop.ins.sync_info = bass_rust.SyncInfo(
                            on_wait=[w], on_update=[])
                        new_list.append(nop.ins)
                        n_split += 1
                    inst.sync_info = bass_rust.SyncInfo(
                        on_wait=waits[:1], on_update=list(si.on_update or []))
                new_list.append(inst)
            if len(new_list) != len(insts):
                insts[:] = new_list
    return n_split


# ---------------------------------------------------------------------------
# Kernel builder
# ---------------------------------------------------------------------------

def build_kernel(debug=False, repeat=1):
    nc = bass.Bass(num_devices=N_CORES)

    x_c = nc.declare_dram_parameter("x_c", [TOK, D], F32, isOutput=False)
    x_cls = nc.declare_dram_parameter("x_cls", [NTOK_CLS, D], F32, isOutput=False)
    w1T = nc.declare_dram_parameter("w1T", [D, H], BF16, isOutput=False)
    w2p = nc.declare_dram_parameter("w2p", [D // 128, 128, H], BF16,
                                    isOutput=False)
    b1p = nc.declare_dram_parameter("b1p", [128, H // 128], F32, isOutput=False)
    b2p = nc.declare_dram_parameter("b2p", [128, D // 128], F32, isOutput=False)
    winp = nc.declare_dram_parameter("winp", [D, NA * HC], BF16, isOutput=False)
    binp = nc.declare_dram_parameter("binp", [128, NA * (HC // 128)], F32,
                                     isOutput=False)
    woutp = nc.declare_dram_parameter("woutp", [NA * (HC // 128), 128, D], BF16,
                                      isOutput=False)
    boutp = nc.declare_dram_parameter("boutp", [1, NA * D], BF16, isOutput=False)
    ghatp = nc.declare_dram_parameter("ghatp", [128, (D // 128) * 2 * NCLS], F32,
                                      isOutput=False)
    bbexp = nc.declare_dram_parameter("bbexp", [NTOK_CLS, 2], F32, isOutput=False)
    y = nc.declare_dram_parameter("y", [TOK, D], F32, isOutput=True)

    if debug:
        p_sh = nc.declare_dram_parameter("p_sh", [KC_ := HC // 128, 128, 2 * NTOK_CLS],
                                         BF16, isOutput=True)
        p_w = nc.declare_dram_parameter("p_w", [2, NTOK_CLS], F32, isOutput=True)
        p_lg = nc.declare_dram_parameter("p_lg", [3, 128, 2], F32, isOutput=True)
        p_cc = nc.declare_dram_parameter("p_cc", [NTOK_CLS, D], F32, isOutput=True)
        p_po = nc.declare_dram_parameter("p_po", [KD_ := D // 128, 128, NTOK_CLS], F32, isOutput=True)
    cc_in = nc.dram_tensor("cc_in", [NTOK_CLS, D], F32)
    cc_out = nc.dram_tensor("cc_out", [NTOK_CLS // N_CORES, D], F32)

    KD = D // 128   # 6 k-tiles over D
    KH = H // 128   # 24 k-tiles over H
    KC = HC // 128  # 3 k-tiles over the per-core hidden slice

    with tile.TileContext(nc) as tc:
        ctx_pool = tc.tile_pool(name="persist", bufs=1)
        with ctx_pool as pp, \
             tc.tile_pool(name="ps_small", bufs=2, space="PSUM") as ps_small, \
             tc.tile_pool(name="stage", bufs=3) as stage:

            ident = pp.tile([128, 128], F32, tag="ident", name="ident")
            make_identity(nc, ident)
            ident16 = pp.tile([128, 128], BF16, tag="ident16", name="ident16")
            make_identity(nc, ident16)

            # ---- persistent SBUF tensors -------------------------------
            w1_sb = [pp.tile([128, H], BF16, tag=f"w1_{k}", name=f"w1_{k}") for k in range(KD)]
            xT_sb = [pp.tile([128, TOK], BF16, tag=f"xT_{k}", name=f"xT_{k}") for k in range(KD)]
            xcT32 = [pp.tile([128, NTOK_CLS], F32, tag=f"xcT32_{k}", name=f"xcT32_{k}")
                     for k in range(KD)]
            xcT16 = [pp.tile([128, NTOK_CLS], BF16, tag=f"xcT16_{k}", name=f"xcT16_{k}")
                     for k in range(KD)]
            win_sb = [pp.tile([128, NA * HC], BF16, tag=f"win_{k}", name=f"win_{k}")
                      for k in range(KD)]
            wout_sb = [pp.tile([128, D], BF16, tag=f"wout_{j}", name=f"wout_{j}")
                       for j in range(NA * KC)]
            SH = [pp.tile([128, 2 * NTOK_CLS], BF16, tag=f"SH_{k}", name=f"SH_{k}")
                  for k in range(KC)]
            b1_sb = pp.tile([128, KH], F32, tag="b1", name="b1")
            b2_sb = pp.tile([128, KD], F32, tag="b2", name="b2")
            bin_sb = pp.tile([128, NA * KC], F32, tag="bin", name="bin")
            bout_sb = pp.tile([1, NA * D], BF16, tag="bout", name="bout")
            ghat_sb = pp.tile([128, KD * 12], F32, tag="ghat", name="ghat")
            ones_sb = pp.tile([1, 128], BF16, tag="ones", name="ones")
            w0T_sb = pp.tile([1, NTOK_CLS], F32, tag="w0T", name="w0T")
            w1T_sb_g = pp.tile([1, NTOK_CLS], F32, tag="w1Tg", name="w1Tg")
            w0T16 = pp.tile([1, NTOK_CLS], BF16, tag="w0T16", name="w0T16")
            w1T16 = pp.tile([1, NTOK_CLS], BF16, tag="w1T16", name="w1T16")
            W0b = pp.tile([128, NTOK_CLS], BF16, tag="W0b", name="W0b")
            W1b = pp.tile([128, NTOK_CLS], BF16, tag="W1b", name="W1b")

            zrow_sb = pp.tile([1, 128], BF16, tag="zrow", name="zrow")
            nc.vector.memset(ones_sb[:, :], 1.0)
            nc.vector.memset(zrow_sb[:, :], 0.0)

            # ---- load weights ------------------------------------------
            for k in range(KD):
                nc.sync.dma_start(out=w1_sb[k][:, :],
                                  in_=w1T[k * 128:(k + 1) * 128, :])
                nc.sync.dma_start(out=win_sb[k][:, :],
                                  in_=winp[k * 128:(k + 1) * 128, :])
            for j in range(NA * KC):
                nc.sync.dma_start(out=wout_sb[j][:, :], in_=woutp[j, :, :])
            nc.sync.dma_start(out=b1_sb[:, :], in_=b1p[:, :])
            nc.sync.dma_start(out=b2_sb[:, :], in_=b2p[:, :])
            nc.sync.dma_start(out=bin_sb[:, :], in_=binp[:, :])
            nc.sync.dma_start(out=bout_sb[:, :], in_=boutp[:, :])
            nc.sync.dma_start(out=ghat_sb[:, :], in_=ghatp[:, :])

            for _rep in range(repeat):
                # ---- phase A1: transpose x -> xT (bf16) --------------------
                n_xtile = (TOK + 127) // 128  # 13
                for i in range(n_xtile):
                    r0 = i * 128
                    rl = min(128, TOK - r0)
                    xr = stage.tile([128, D], F32, tag="xrow", name="xrow")
                    nc.sync.dma_start(out=xr[:rl, :], in_=x_c[r0:r0 + rl, :])
                    xr16 = stage.tile([128, D], BF16, tag="xrow16", name="xrow16")
                    nc.vector.tensor_copy(xr16[:rl, :], xr[:rl, :])
                    for k in range(KD):
                        pt = ps_small.tile([128, 128], BF16, tag="tp", name="tp16")
                        nc.tensor.transpose(pt[:, :rl], xr16[:rl, k * 128:(k + 1) * 128],
                                            ident16[:rl, :rl])
                        nc.vector.tensor_copy(xT_sb[k][:, r0:r0 + rl], pt[:, :rl])

                # ---- phase A2: cls tokens, gate ----------------------------
                with tc.tile_pool(name="cls_tmp", bufs=2) as cls_tmp, \
                     tc.tile_pool(name="ps_mm", bufs=2, space="PSUM") as ps_mm:
                    for i in range(3):  # 3 tiles of 128 cls tokens, (t,b) order
                        xc = cls_tmp.tile([128, D], F32, tag="xc", name="xc")
                        nc.sync.dma_start(out=xc[:, :],
                                          in_=x_cls[i * 128:(i + 1) * 128, :])
                        sq = cls_tmp.tile([128, D], F32, tag="sq", name="sq")
                        nsq = cls_tmp.tile([128, 1], F32, tag="nsq", name="nsq")
                        nc.scalar.activation(sq[:, :], xc[:, :], AF.Square,
                                             accum_out=nsq[:, :])
                        nrm = cls_tmp.tile([128, 1], F32, tag="nrm", name="nrm")
                        nc.scalar.activation(nrm[:, :], nsq[:, :], AF.Sqrt)
                        rn = cls_tmp.tile([128, 1], F32, tag="rn", name="rn")
                        nc.vector.reciprocal(rn[:, :], nrm[:, :])
                        # transpose raw cls tile -> xcT32 / xcT16
                        for k in range(KD):
                            pt = ps_small.tile([128, 128], F32, tag="tp", name="tp")
                            nc.tensor.transpose(pt[:, :], xc[:, k * 128:(k + 1) * 128],
                                                ident[:, :])
                            nc.vector.tensor_copy(xcT32[k][:, i * 128:(i + 1) * 128], pt[:, :])
                            nc.vector.tensor_copy(xcT16[k][:, i * 128:(i + 1) * 128],
                                                  pt[:, :])

                        # gate logits: [tok,12] = xclsT.T @ ghat (fp32), × 1/|x|
                        pg = ps_mm.tile([128, 12], F32, tag="ph", name="pg")
                        for k in range(KD):
                            nc.tensor.matmul(
                                pg[:, :],
                                lhsT=xcT32[k][:, i * 128:(i + 1) * 128],
                                rhs=ghat_sb[:, k * 12:(k + 1) * 12],
                                start=(k == 0), stop=(k == KD - 1))
                        lg = cls_tmp.tile([128, 12], F32, tag="lg", name="lg")
                        nc.vector.tensor_scalar_mul(lg[:, :], pg[:, :], rn[:, :])

                        bb_sb = cls_tmp.tile([128, 2], F32, tag="bb", name="bb")
                        nc.sync.dma_start(out=bb_sb[:, :],
                                          in_=bbexp[i * 128:(i + 1) * 128, :])
                        d01 = cls_tmp.tile([128, 2], F32, tag="d01", name="d01")
                        # rows [0:64] are task 2i, rows [64:128] task 2i+1
                        t0, t1 = 2 * i, 2 * i + 1
                        nc.vector.tensor_tensor(d01[0:64, :], lg[0:64, 2 * t0:2 * t0 + 2],
                                                bb_sb[0:64, :], ALU.add)
                        nc.vector.tensor_tensor(d01[64:128, :],
                                                lg[64:128, 2 * t1:2 * t1 + 2],
                                                bb_sb[64:128, :], ALU.add)
                        if debug:
                            nc.sync.dma_start(out=p_lg[i, :, :], in_=d01[:, :])
                        diff = cls_tmp.tile([128, 1], F32, tag="diff", name="diff")
                        nc.vector.tensor_tensor(diff[:, :], d01[:, 0:1], d01[:, 1:2],
                                                ALU.subtract)
                        ad = cls_tmp.tile([128, 1], F32, tag="ad", name="ad")
                        nc.scalar.activation(ad[:, :], diff[:, :], AF.Abs)
                        pmax = cls_tmp.tile([128, 1], F32, tag="pmax", name="pmax")
                        nc.scalar.activation(pmax[:, :], ad[:, :], AF.Sigmoid)
                        m0 = cls_tmp.tile([128, 1], F32, tag="m0", name="m0")
                        nc.vector.tensor_scalar(m0[:, :], diff[:, :], 0.0, None,
                                                ALU.is_ge)
                        w0 = cls_tmp.tile([128, 1], F32, tag="w0", name="w0")
                        nc.vector.tensor_tensor(w0[:, :], m0[:, :], pmax[:, :],
                                                ALU.mult)
                        w1g = cls_tmp.tile([128, 1], F32, tag="w1g", name="w1g")
                        nc.vector.tensor_tensor(w1g[:, :], pmax[:, :], w0[:, :],
                                                ALU.subtract)
                        # transpose w0/w1 -> row vectors
                        ptw = ps_small.tile([128, 128], F32, tag="tp", name="tp")
                        nc.tensor.transpose(ptw[:1, :], w0[:, 0:1], ident[:, :])
                        nc.vector.tensor_copy(w0T_sb[:, i * 128:(i + 1) * 128], ptw[:1, :])
                        ptw2 = ps_small.tile([128, 128], F32, tag="tp", name="tp")
                        nc.tensor.transpose(ptw2[:1, :], w1g[:, 0:1], ident[:, :])
                        nc.vector.tensor_copy(w1T_sb_g[:, i * 128:(i + 1) * 128], ptw2[:1, :])

                    nc.vector.tensor_copy(w0T16[:, :], w0T_sb[:, :])
                    nc.vector.tensor_copy(w1T16[:, :], w1T_sb_g[:, :])

                    # broadcast w0/w1 across partitions, in SH column order.
                    # SH r0-cols: [t0,t2,t4 | t1,t3,t5] (by dst atom 3 then 4)
                    pw = ps_mm.tile([128, NTOK_CLS], F32, tag="ph", name="pw")
                    ev = w0T16.rearrange("p (t b) -> p t b", b=64)
                    nc.tensor.matmul(pw[:, 0:192], lhsT=ones_sb[:, :],
                                     rhs=ev[:, 0:6:2, :], start=True, stop=True)
                    nc.tensor.matmul(pw[:, 192:384], lhsT=ones_sb[:, :],
                                     rhs=ev[:, 1:6:2, :], start=True, stop=True)
                    nc.vector.tensor_copy(W0b[:, :], pw[:, :])
                    pw2 = ps_mm.tile([128, NTOK_CLS], F32, tag="ph", name="pw")
                    nc.tensor.matmul(pw2[:, :], lhsT=ones_sb[:, :],
                                     rhs=w1T16[:, :], start=True, stop=True)
                    nc.vector.tensor_copy(W1b[:, :], pw2[:, :])

                    # ---- phase C: atom stage-1 (hid in SH layout) ----------
                    # SH col layout: [dst3: t0,t2,t4 (192)][dst4: t1,t3,t5 (192)]
                    #                [dst0: t0,t1 (128)][dst1: t2,t3][dst2: t4,t5]
                    xv = [xcT16[k].rearrange("p (t b) -> p t b", b=64)
                          for k in range(KD)]
                    for a in range(NA):
                        for m in range(KC):
                            # only the tokens whose tasks use atom a as src
                            na = 128 if a <= 2 else 192
                            ph = ps_mm.tile([128, NTOK_CLS], F32, tag="ph", name="ph")
                            for k in range(KD):
                                if a <= 2:
                                    rhs = xcT16[k][:, a * 128:(a + 1) * 128]
                                else:
                                    rhs = xv[k][:, (a - 3):NCLS:2, :]
                                nc.tensor.matmul(
                                    ph[:, :na],
                                    lhsT=win_sb[k][:, a * HC + m * 128:
                                                   a * HC + (m + 1) * 128],
                                    rhs=rhs,
                                    start=(k == 0), stop=(k == KD - 1))
                            bias = bin_sb[:, a * KC + m: a * KC + m + 1]
                            if a <= 2:
                                # cols: task 2a then 2a+1 (r0 -> dst3/dst4 blks)
                                nc.scalar.activation(
                                    SH[m][:, a * 64:(a + 1) * 64],
                                    ph[:, 0:64], AF.Gelu, bias=bias)
                                nc.scalar.activation(
                                    SH[m][:, 192 + a * 64:192 + (a + 1) * 64],
                                    ph[:, 64:128], AF.Gelu, bias=bias)
                            else:
                                # cols: even (a=3) / odd (a=4) tasks in order
                                off = 64 * (a - 3)
                                for g in range(3):
                                    nc.scalar.activation(
                                        SH[m][:, 384 + g * 128 + off:
                                              384 + g * 128 + off + 64],
                                        ph[:, g * 64:(g + 1) * 64],
                                        AF.Gelu, bias=bias)
                    # scale: r0 cols by w0 (col-permuted), r1 cols by w1
                    for m in range(KC):
                        nc.vector.tensor_tensor(SH[m][:, 0:384], SH[m][:, 0:384],
                                                W0b[:, :], ALU.mult)
                        nc.vector.tensor_tensor(SH[m][:, 384:768], SH[m][:, 384:768],
                                                W1b[:, :], ALU.mult)
                    if debug:
                        for m in range(KC):
                            nc.sync.dma_start(out=p_sh[m, :, :], in_=SH[m][:, :])
                        nc.sync.dma_start(out=p_w[0:1, :], in_=w0T_sb[:, :])
                        nc.sync.dma_start(out=p_w[1:2, :], in_=w1T_sb_g[:, :])

                # ---- phase D: atom stage-2 into partial cls out ------------
                with tc.tile_pool(name="ps_out", bufs=1, space="PSUM") as ps_out, \
                     tc.tile_pool(name="fin", bufs=2) as fin:
                    pouts = [ps_out.tile([128, NTOK_CLS], F32, tag=f"po_{dp}", name=f"po_{dp}")
                             for dp in range(KD)]
                    shr = [SH[k].rearrange("p (q b) -> p q b", b=64)
                           for k in range(KC)]
                    for dp in range(KD):
                        # hw-clear + zero the whole tile once; then accumulate-only.
                        # (start=True clears PSUM has_written at bank granularity,
                        # so it must appear exactly once per bank.)
                        nc.tensor.matmul(pouts[dp][:, :], lhsT=zrow_sb[:, :],
                                         rhs=W0b[:1, :], start=True, stop=False)
                        # r0: dst atom 3 (cols t0,t2,t4), dst atom 4 (t1,t3,t5)
                        for ai, a in enumerate((3, 4)):
                            po = pouts[dp].rearrange("p (t b) -> p t b", b=64)
                            out_ap = po[:, ai:NCLS:2, :]
                            for k in range(KC):
                                nc.tensor.matmul(
                                    out_ap,
                                    lhsT=wout_sb[a * KC + k][:, dp * 128:(dp + 1) * 128],
                                    rhs=shr[k][:, 3 * ai:3 * (ai + 1), :],
                                    start=False, stop=False)
                        # r1: dst atoms 0,1,2 (cols t2a, t2a+1)
                        for a in range(3):
                            out_ap = pouts[dp][:, a * 128:(a + 1) * 128]
                            for k in range(KC):
                                nc.tensor.matmul(
                                    out_ap,
                                    lhsT=wout_sb[a * KC + k][:, dp * 128:(dp + 1) * 128],
                                    rhs=SH[k][:, 384 + a * 128:384 + (a + 1) * 128],
                                    start=False, stop=False)
                        # bias rows (atom_out_b/8), weighted by w0/w1
                        w0r = w0T16.rearrange("p (t b) -> p t b", b=64)
                        for ai, a in enumerate((3, 4)):
                            po = pouts[dp].rearrange("p (t b) -> p t b", b=64)
                            nc.tensor.matmul(
                                po[:, ai:NCLS:2, :],
                                lhsT=bout_sb[:, a * D + dp * 128:a * D + (dp + 1) * 128],
                                rhs=w0r[:, ai:NCLS:2, :],
                                start=False, stop=False)
                        for a in range(3):
                            nc.tensor.matmul(
                                pouts[dp][:, a * 128:(a + 1) * 128],
                                lhsT=bout_sb[:, a * D + dp * 128:a * D + (dp + 1) * 128],
                                rhs=w1T16[:, a * 128:(a + 1) * 128],
                                start=False, stop=True)

                    # ---- phase E: transpose partial, ReduceScatter, store --
                    pt_sb = [fin.tile([128, NTOK_CLS], F32, tag=f"pt_{dp}", name=f"pt_{dp}")
                             for dp in range(KD)]
                    for dp in range(KD):
                        nc.vector.tensor_copy(pt_sb[dp][:, :], pouts[dp][:, :])
                    if debug:
                        for dp in range(KD):
                            nc.sync.dma_start(out=p_po[dp, :, :], in_=pt_sb[dp][:, :])
                    for i in range(3):  # token blocks of 128 (t,b order)
                        o3 = fin.tile([128, D], F32, tag="o3", name="o3")
                        for dp in range(KD):
                            ptt = ps_small.tile([128, 128], F32, tag="tp", name="tp")
                            nc.tensor.transpose(ptt[:, :],
                                                pt_sb[dp][:, i * 128:(i + 1) * 128],
                                                ident[:, :])
                            nc.vector.tensor_copy(o3[:, dp * 128:(dp + 1) * 128], ptt[:, :])
                        # rows are tokens (t,b): t = 2i + (row>=64), b = row%64
                        # cc_in row index = b*6 + t
                        for half in range(2):
                            t = 2 * i + half
                            cc_view = cc_in.rearrange("(b t) d -> b t d", t=NCLS)
                            nc.sync.dma_start(
                                out=cc_view[:, t, :],
                                in_=o3[half * 64:(half + 1) * 64, :])

                    if debug:
                        nc.sync.dma_start(out=p_cc[:, :], in_=cc_in[:, :])
                    nc.gpsimd.collective_compute(
                        "ReduceScatter", ALU.add,
                        replica_groups=[list(range(N_CORES))],
                        ins=[cc_in[:, :]], outs=[cc_out[:, :]])

                    # cc_out rows: (b, t) for this core's 8 batches -> y rows b*T+t
                    y_view = y.rearrange("(b t) d -> b t d", t=T)
                    nc.sync.dma_start(out=y_view[:, 0:NCLS, :], in_=cc_out[:, :])
                # ---- phase B: patch MLP ------------------------------------
                with tc.tile_pool(name="mlp", bufs=1) as mp, \
                     tc.tile_pool(name="w2s", bufs=3) as w2s, \
                     tc.tile_pool(name="yt", bufs=2) as ytp, \
                     tc.tile_pool(name="ps_mm2", bufs=6, space="PSUM") as ps_mm2:
                    PTT = 2 * NP  # 394 patch tokens per tile (2 batches)
                    xvw = [xT_sb[k].rearrange("p (b t) -> p b t", t=T)
                           for k in range(KD)]
                    for tt in range(TT):
                        hT = [mp.tile([128, PTT], BF16, tag=f"hT_{j}", name=f"hT_{j}")
                              for j in range(KH)]
                        for h in range(KH):
                            ph = ps_mm2.tile([128, PTT], F32, tag="pmm", name="pmm")
                            for k in range(KD):
                                nc.tensor.matmul(
                                    ph[:, :],
                                    lhsT=w1_sb[k][:, h * 128:(h + 1) * 128],
                                    rhs=xvw[k][:, 2 * tt:2 * tt + 2, NCLS:T],
                                    start=(k == 0), stop=(k == KD - 1))
                            nc.scalar.activation(hT[h][:, :], ph[:, :], AF.Gelu,
                                                 bias=b1_sb[:, h:h + 1])
                        yT = [mp.tile([128, PTT], F32, tag=f"yT_{j}", name=f"yT_{j}")
                              for j in range(KD)]
                        for dp in range(KD):
                            ph2 = ps_mm2.tile([128, PTT], F32, tag="pmm", name="pmm")
                            wt = w2s.tile([128, KH * 128], BF16, tag="w2dp", name="w2dp")
                            nc.sync.dma_start(out=wt[:, :], in_=w2p[dp, :, :])
                            for k in range(KH):
                                nc.tensor.matmul(ph2[:, :],
                                                 lhsT=wt[:, k * 128:(k + 1) * 128],
                                                 rhs=hT[k][:, :],
                                                 start=(k == 0), stop=(k == KH - 1))
                            nc.scalar.activation(yT[dp][:, :], ph2[:, :], AF.Identity,
                                                 bias=b2_sb[:, dp:dp + 1])
                        # transpose back to [tok, d] and store (pure patch rows)
                        nblk = (PTT + 127) // 128
                        for bkl in range(nblk):
                            l0 = bkl * 128
                            bl = min(128, PTT - l0)
                            gs = tt * PTT + l0  # global patch index
                            yt_sb = ytp.tile([128, D], F32, tag="ytok", name="ytok")
                            for dp in range(KD):
                                ptt = ps_small.tile([128, 128], F32, tag="tp", name="tp")
                                nc.tensor.transpose(ptt[:bl, :], yT[dp][:, l0:l0 + bl],
                                                    ident[:, :])
                                nc.vector.tensor_copy(yt_sb[:bl, dp * 128:(dp + 1) * 128],
                                               ptt[:bl, :])
                            r = gs
                            end = gs + bl
                            while r < end:
                                b_i, p_i = divmod(r, NP)
                                nxt = min(end, (b_i + 1) * NP)
                                ys = b_i * T + NCLS + p_i
                                nc.sync.dma_start(
                                    out=y[ys:ys + (nxt - r), :],
                                    in_=yt_sb[r - gs:nxt - gs, :])
                                r = nxt


    legalize_sync_waits(nc)
    return nc


# ---------------------------------------------------------------------------
# Host side
# ---------------------------------------------------------------------------

_CACHE = {}


def _prep_inputs(x, fc1_w, fc1_b, fc2_w, fc2_b, gate_pair, atom_in_w, atom_in_b,
                 atom_out_w, atom_out_b, balance_bias):
    bf = ml_dtypes.bfloat16
    x = np.asarray(x, np.float32)
    common = {
        "w1T": np.ascontiguousarray(np.asarray(fc1_w, np.float32).T).astype(bf),
        "w2p": np.ascontiguousarray(
            np.asarray(fc2_w, np.float32).T.reshape(H // 128, 128, D // 128, 128)
            .transpose(2, 1, 0, 3).reshape(D // 128, 128, H)).astype(bf),
        "b1p": np.ascontiguousarray(
            np.asarray(fc1_b, np.float32).reshape(H // 128, 128).T),
        "b2p": np.ascontiguousarray(
            np.asarray(fc2_b, np.float32).reshape(D // 128, 128).T),
        "boutp": (np.asarray(atom_out_b, np.float32) / N_CORES)
            .reshape(1, NA * D).astype(bf),
        "bbexp": np.repeat(np.asarray(balance_bias, np.float32), B, axis=0)
            .reshape(NTOK_CLS, 2),
    }
    g = np.asarray(gate_pair, np.float32)
    gn = g / np.clip(np.linalg.norm(g, axis=-1, keepdims=True), 1e-12, None)
    ghatT = gn.reshape(2 * NCLS, D).T  # [D, 12]
    common["ghatp"] = np.ascontiguousarray(
        ghatT.reshape(KD_ := D // 128, 128, 2 * NCLS)
        .transpose(1, 0, 2).reshape(128, KD_ * 2 * NCLS))
    # cls tokens for all batches in (t, b) order
    xc = np.asarray(x[:, :NCLS, :], np.float32)  # [B, 6, D]
    common["x_cls"] = np.ascontiguousarray(
        xc.transpose(1, 0, 2).reshape(NTOK_CLS, D))

    aiw = np.asarray(atom_in_w, np.float32)   # [5, H, D]
    aib = np.asarray(atom_in_b, np.float32)   # [5, H]
    aow = np.asarray(atom_out_w, np.float32)  # [5, D, H]

    in_maps = []
    for c in range(N_CORES):
        hs = slice(c * HC, (c + 1) * HC)
        m = dict(common)
        m["x_c"] = np.ascontiguousarray(
            x[c * BC:(c + 1) * BC].reshape(TOK, D))
        m["winp"] = np.ascontiguousarray(
            aiw[:, hs, :].transpose(2, 0, 1).reshape(D, NA * HC)).astype(bf)
        m["binp"] = np.ascontiguousarray(
            aib[:, hs].reshape(NA, HC // 128, 128).transpose(2, 0, 1)
            .reshape(128, NA * (HC // 128)))
        m["woutp"] = np.ascontiguousarray(
            aow[:, :, hs].transpose(0, 2, 1)
            .reshape(NA, HC // 128, 128, D)
            .reshape(NA * (HC // 128), 128, D)).astype(bf)
        in_maps.append(m)
    return in_maps


def _get_nc():
    if "nc" not in _CACHE:
        _CACHE["nc"] = build_kernel()
    return _CACHE["nc"]


def kernel(**inputs) -> np.ndarray:
    nc = _get_nc()
    in_maps = _prep_inputs(**inputs)
    res = run_bass_kernel_spmd(nc, in_maps, core_ids=list(range(N_CORES)))
    out = np.empty((B, T, D), np.float32)
    for c in range(N_CORES):
        out[c * BC:(c + 1) * BC] = res.results[c]["y"].reshape(BC, T, D)
    return out


if __name__ == "__main__":
    nc = build_kernel()
    n = sum(len(bb.instructions) for f in nc.m.functions for bb in f.blocks)
    print("instructions:", n)

